# revision 17
# baseline (speedup 1.0000x reference)
"""Paged-attention decode (GQA) on 8 Trainium2 NeuronCores.

Sharding: tensor-parallel over heads. Core c owns KV head c (KVH=8) and the
4 query heads in its GQA group. The KV cache / new K/V / query are sliced
per-core on the host (pure shard along the KV-head dim); block_tables and
seq_lens are folded into the compiled graph (decode launch config). Each core
runs an identical SPMD graph with no collectives; the host concatenates the
per-core output slices.

Device algorithm per core, per sequence b (L = seq_lens[b], Lc = L-1 cached
tokens, tiles of 128 tokens):
  - DMA K/V tiles (f32 HBM -> bf16 SBUF cast in the SWDGE datapath)
  - PE transpose each K tile -> KT [d, t]
  - scoresT[t, 4] = KT.T-matmul with qT[d, 4] (per 128-token tile, into one
    PSUM bank per sequence), plus a 1-row slot for the new token's score
  - exp(scale*s) on ACT (PSUM -> bf16 SBUF probsT), mask tail rows / new-token
    rows by memset-0 (softmax-without-max: scores are O(5), no overflow)
  - out^T[d, 4] += V_tile.T-matmul probsT tile slices, accumulated in PSUM,
    plus a rank-1 update with v_new
  - denominator l = ones-matmul over probsT, reduced per sequence on DVE
  - finalize: broadcast 1/l via a rank-1 matmul, multiply, PE-transpose to
    [(b,g), d] layout, DMA out.
"""

import numpy as np
import sys

for _p in ("/opt/trn_rl_repo",):
    if _p not in sys.path:
        sys.path.append(_p)

SCALE = 0.08838834764831845
P = 128  # partition / head-dim / token-tile size


def _build_graph(
    nt, rem, n_tokens, s_max, dma_only=False, pipeline_pv=True, replay=1
):
    """Build the SPMD Bacc graph, specialized on per-seq tile counts.

    nt[b]  = number of 128-token cache tiles for seq b
    rem[b] = valid tokens in the last tile (1..128), 0 if nt[b] == 0
    n_tokens = rows of the per-core flat cache input (B * s_max)
    s_max  = tokens per sequence in the flat cache layout
    dma_only = ablation: issue only the K/V loads (timing the memory floor)
    pipeline_pv = emit seq b's PV phase after seq b+1's score phase, so the
        exp round-trip through ScalarE doesn't stall the PE stream
    """
    import concourse.bass as bass  # noqa: F401
    import concourse.mybir as mybir
    import concourse.tile as tile
    from concourse import bacc
    from concourse.masks import make_identity

    B = len(nt)
    G = 4  # query heads per core
    MAXS = int(max(nt)) + 1  # max slots (tiles + new-token) per seq
    f32 = mybir.dt.float32
    bf16 = mybir.dt.bfloat16

    nc = bacc.Bacc(None, target_bir_lowering=False)
    kc = nc.dram_tensor("kc", [n_tokens, P], f32, kind="ExternalInput")
    vc = nc.dram_tensor("vc", [n_tokens, P], f32, kind="ExternalInput")
    qh = nc.dram_tensor("qh", [P, B * G], f32, kind="ExternalInput")  # [d,(b,g)]
    kn = nc.dram_tensor("kn", [P, B], f32, kind="ExternalInput")  # [d, b]
    vn = nc.dram_tensor("vn", [1, B * P], f32, kind="ExternalInput")  # [1,(b,d)]
    out = nc.dram_tensor("out", [B, G * P], f32, kind="ExternalOutput")

    with tile.TileContext(nc) as tc:
        with tc.tile_pool(name="persist", bufs=1) as persist:
            ident_bf = persist.tile([P, P], bf16)
            make_identity(nc, ident_bf)
            ident_f = persist.tile([P, P], f32)
            make_identity(nc, ident_f)
            ones_col_bf = persist.tile([P, 1], bf16)
            nc.vector.memset(ones_col_bf, 1.0)
            ones_row_f = persist.tile([1, P], f32)
            nc.vector.memset(ones_row_f, 1.0)
            # mask_tab[p, r] = 1.0 if p < r else 0.0 — per-partition masks for
            # the partial last tile (r = rem) and the new-token slot (r = 1)
            mask_tab = persist.tile([P, P + 1], f32)
            nc.gpsimd.memset(mask_tab, 0.0)
            # out[p, r] = (p - r >= 0) ? 0.0 (in_) : 1.0 (fill)  ==  p < r
            nc.gpsimd.affine_select(
                out=mask_tab,
                in_=mask_tab,
                compare_op=mybir.AluOpType.is_ge,
                fill=1.0,
                base=0,
                pattern=[[-1, P + 1]],
                channel_multiplier=1,
            )
            qh_bf = persist.tile([P, B * G], bf16)
            nc.gpsimd.dma_start(qh_bf[:], qh[:])
            kn_bf = persist.tile([P, B], bf16)
            nc.gpsimd.dma_start(kn_bf[:], kn[:])
            vn_bf = persist.tile([1, B * P], bf16)
            nc.gpsimd.dma_start(vn_bf[:], vn[:])
            outT = persist.tile([P, B * G], f32)  # [d, (b,g)]
            l_red = persist.tile([1, B * G], f32)
            recip = persist.tile([1, B * G], f32)
            outN = persist.tile([P, B * G], f32)
            outF = persist.tile([P, B * G], f32)

            with (
                tc.tile_pool(name="kv", bufs=3) as kvpool,
                tc.tile_pool(name="kt_ps", bufs=2, space="PSUM") as ktps,
                tc.tile_pool(name="kt_sb", bufs=3) as ktsb,
                tc.tile_pool(name="sc_ps", bufs=2, space="PSUM") as scps,
                tc.tile_pool(name="probs", bufs=2) as prpool,
                tc.tile_pool(name="ot_ps", bufs=2, space="PSUM") as otps,
                tc.tile_pool(name="l_ps", bufs=2, space="PSUM") as lps,
            ):
                state = {}

                def emit_load(b):
                    ntb = int(nt[b])
                    tok_off = b * s_max
                    kb = vb = None
                    if ntb > 0:
                        kb = kvpool.tile([P, MAXS - 1, P], bf16, tag="K")
                        vb = kvpool.tile([P, MAXS - 1, P], bf16, tag="V")
                        src_k = kc[tok_off : tok_off + ntb * P, :].rearrange(
                            "(o p) d -> p o d", p=P
                        )
                        src_v = vc[tok_off : tok_off + ntb * P, :].rearrange(
                            "(o p) d -> p o d", p=P
                        )
                        nc.gpsimd.dma_start(kb[:, :ntb, :], src_k)
                        nc.gpsimd.dma_start(vb[:, :ntb, :], src_v)
                    return kb, vb

                def emit_scores(b, kb, vb):
                    ntb = int(nt[b])
                    ns = ntb + 1
                    scores = scps.tile([P, G * MAXS], f32)
                    if b < 2:
                        # scrub pre-kernel PSUM garbage in the two rotating
                        # score buffers (rows the new-token slot never writes)
                        nc.vector.memset(scores, 0.0)
                    for i in range(ntb):
                        ktp = ktps.tile([P, P], bf16)
                        nc.tensor.transpose(ktp, kb[:, i, :], ident_bf)
                        kts = ktsb.tile([P, P], bf16)
                        if i % 2 == 0:
                            nc.vector.tensor_copy(kts, ktp)
                        else:
                            nc.scalar.copy(kts, ktp)
                        nc.tensor.matmul(
                            scores[:, G * i : G * (i + 1)],
                            lhsT=kts,
                            rhs=qh_bf[:, G * b : G * (b + 1)],
                            start=True,
                            stop=True,
                        )
                    # new-token score (row 0 of its slot)
                    nc.tensor.matmul(
                        scores[0:1, G * ntb : G * ns],
                        lhsT=kn_bf[:, b : b + 1],
                        rhs=qh_bf[:, G * b : G * (b + 1)],
                        start=True,
                        stop=True,
                    )
                    pb = prpool.tile([P, G * MAXS], bf16)
                    nc.scalar.activation(
                        pb[:, : G * ns],
                        scores[:, : G * ns],
                        mybir.ActivationFunctionType.Exp,
                        scale=SCALE,
                    )
                    if ntb > 0 and rem[b] < P:
                        r = int(rem[b])
                        nc.vector.tensor_scalar_mul(
                            pb[:, G * (ntb - 1) : G * ntb],
                            pb[:, G * (ntb - 1) : G * ntb],
                            mask_tab[:, r : r + 1],
                        )
                    nc.vector.tensor_scalar_mul(
                        pb[:, G * ntb : G * ns],
                        pb[:, G * ntb : G * ns],
                        mask_tab[:, 1:2],
                    )
                    state[b] = (pb, vb)

                def emit_pv(b):
                    ntb = int(nt[b])
                    ns = ntb + 1
                    pb, vb = state.pop(b)
                    lp = lps.tile([1, G * MAXS], f32)
                    nc.tensor.matmul(
                        lp[:, : G * ns],
                        lhsT=ones_col_bf,
                        rhs=pb[:, : G * ns],
                        start=True,
                        stop=True,
                    )
                    otp = otps.tile([P, G], f32)
                    for i in range(ntb):
                        nc.tensor.matmul(
                            otp,
                            lhsT=vb[:, i, :],
                            rhs=pb[:, G * i : G * (i + 1)],
                            start=(i == 0),
                            stop=False,
                        )
                    nc.tensor.matmul(
                        otp,
                        lhsT=vn_bf[0:1, P * b : P * (b + 1)],
                        rhs=pb[0:1, G * ntb : G * ns],
                        start=(ntb == 0),
                        stop=True,
                    )
                    nc.vector.tensor_copy(outT[:, G * b : G * (b + 1)], otp)
                    nc.vector.tensor_reduce(
                        l_red[0:1, G * b : G * (b + 1)],
                        lp[0:1, : G * ns].rearrange("p (i h) -> p h i", h=G),
                        axis=mybir.AxisListType.X,
                        op=mybir.AluOpType.add,
                    )

                def emit_body():
                    if dma_only:
                        for b in range(B):
                            kb, vb = emit_load(b)
                            if kb is not None:
                                # tiny consumers so the loads aren't dead
                                nc.vector.tensor_copy(
                                    outT[0:1, G * b : G * b + 1],
                                    kb[0:1, 0, 0:1],
                                )
                                nc.vector.tensor_copy(
                                    outT[0:1, G * b + 1 : G * b + 2],
                                    vb[0:1, 0, 0:1],
                                )
                        nc.vector.memset(l_red, 1.0)
                    elif pipeline_pv:
                        prev = None
                        for b in range(B):
                            kb, vb = emit_load(b)
                            emit_scores(b, kb, vb)
                            if prev is not None:
                                emit_pv(prev)
                            prev = b
                        emit_pv(prev)
                    else:
                        for b in range(B):
                            kb, vb = emit_load(b)
                            emit_scores(b, kb, vb)
                            emit_pv(b)

                if replay > 1:
                    with tc.For_i(0, replay, 1):
                        emit_body()
                else:
                    emit_body()

            # ---- finalize: out = outT / l, transposed to [(b,g), d] ----
            with tc.tile_pool(name="fin_ps", bufs=1, space="PSUM") as finps:
                nc.vector.reciprocal(recip, l_red)
                bc = finps.tile([P, B * G], f32)
                nc.tensor.matmul(
                    bc, lhsT=ones_row_f, rhs=recip, start=True, stop=True
                )
                nc.vector.tensor_mul(outN, outT, bc)
                tp2 = finps.tile([P, B * G], f32)
                nc.tensor.transpose(tp2, outN, ident_f)
                nc.vector.tensor_copy(outF, tp2)
                nc.sync.dma_start(
                    out.rearrange("b (g d) -> (b g) d", g=G), outF
                )
    nc.compile()
    return nc


def _prepare(query, key, value, key_cache, value_cache, block_tables, seq_lens):
    """Build the compiled SPMD graph and the per-core input shards."""
    query = np.ascontiguousarray(np.asarray(query, dtype=np.float32))
    key = np.ascontiguousarray(np.asarray(key, dtype=np.float32))
    value = np.ascontiguousarray(np.asarray(value, dtype=np.float32))
    key_cache = np.asarray(key_cache, dtype=np.float32)
    value_cache = np.asarray(value_cache, dtype=np.float32)
    block_tables = np.asarray(block_tables)
    seq_lens = np.asarray(seq_lens)

    B, H, D = query.shape
    KVH = key.shape[1]
    NB, BS = key_cache.shape[0], key_cache.shape[1]
    S_MAX = block_tables.shape[1] * BS
    G = H // KVH
    N_CORES = 8
    assert KVH == N_CORES and D == P

    L = np.maximum(seq_lens.astype(np.int64), 1)
    Lc = L - 1  # cache tokens attended (position L-1 comes from k/v_new)
    nt = ((Lc + P - 1) // P).astype(np.int64)
    rem = Lc - np.maximum(nt - 1, 0) * P  # valid tokens in last tile

    kc_flat = key_cache.reshape(NB * BS, KVH, D)
    vc_flat = value_cache.reshape(NB * BS, KVH, D)

    # The flat per-core cache is laid out seq-major: token t of seq b at row
    # b*S_MAX + t. With arange block tables (the spec's fill) that is exactly
    # the cache's own layout — a pure KV-head shard. Otherwise resolve the
    # paged layout with a host gather.
    arange_ok = bool(
        np.array_equal(
            block_tables.ravel(),
            np.arange(block_tables.size, dtype=block_tables.ravel().dtype),
        )
    )
    if not arange_ok:
        t = np.arange(S_MAX, dtype=np.int64)
        gather_idx = (
            block_tables[:, t // BS].astype(np.int64) * BS + t % BS
        ).reshape(-1)

    nc = _build_graph(nt, rem, B * S_MAX, S_MAX)

    in_maps = []
    for c in range(N_CORES):
        if arange_ok:
            kc_c = np.ascontiguousarray(kc_flat[:, c, :])
            vc_c = np.ascontiguousarray(vc_flat[:, c, :])
        else:
            kc_c = np.ascontiguousarray(kc_flat[gather_idx, c, :])
            vc_c = np.ascontiguousarray(vc_flat[gather_idx, c, :])
        qh_c = np.ascontiguousarray(
            query[:, c * G : (c + 1) * G, :].transpose(2, 0, 1).reshape(D, B * G)
        )
        kn_c = np.ascontiguousarray(key[:, c, :].T)
        vn_c = np.ascontiguousarray(value[:, c, :].reshape(1, B * D))
        in_maps.append(
            {"kc": kc_c, "vc": vc_c, "qh": qh_c, "kn": kn_c, "vn": vn_c}
        )
    return nc, in_maps, (B, H, D, G)


def kernel(query, key, value, key_cache, value_cache, block_tables, seq_lens):
    from concourse.bass_utils import run_bass_kernel_spmd

    nc, in_maps, (B, H, D, G) = _prepare(
        query, key, value, key_cache, value_cache, block_tables, seq_lens
    )
    res = run_bass_kernel_spmd(nc, in_maps, core_ids=list(range(len(in_maps))))
    out = np.empty((B, H * D), np.float32)
    for c in range(len(in_maps)):
        out[:, c * G * D : (c + 1) * G * D] = res.results[c]["out"]
    return out


# revision 20
# speedup vs baseline: 1.0983x; 1.0983x over previous
"""Paged-attention decode (GQA) on 8 Trainium2 NeuronCores.

Sharding: tensor-parallel over heads. Core c owns KV head c (KVH=8) and the
4 query heads in its GQA group. The KV cache / new K/V / query are sliced
per-core on the host (pure shard along the KV-head dim); block_tables and
seq_lens are folded into the compiled graph (decode launch config). Each core
runs an identical SPMD graph with no collectives; the host concatenates the
per-core output slices.

Device algorithm per core, per sequence b (L = seq_lens[b], Lc = L-1 cached
tokens, tiles of 128 tokens):
  - DMA K/V tiles (f32 HBM -> bf16 SBUF cast in the SWDGE datapath)
  - PE transpose each K tile -> KT [d, t]
  - scoresT[t, 4] = KT.T-matmul with qT[d, 4] (per 128-token tile, into one
    PSUM bank per sequence), plus a 1-row slot for the new token's score
  - exp(scale*s) on ACT (PSUM -> bf16 SBUF probsT), mask tail rows / new-token
    rows by memset-0 (softmax-without-max: scores are O(5), no overflow)
  - out^T[d, 4] += V_tile.T-matmul probsT tile slices, accumulated in PSUM,
    plus a rank-1 update with v_new
  - denominator l = ones-matmul over probsT, reduced per sequence on DVE
  - finalize: broadcast 1/l via a rank-1 matmul, multiply, PE-transpose to
    [(b,g), d] layout, DMA out.
"""

import numpy as np
import sys

for _p in ("/opt/trn_rl_repo",):
    if _p not in sys.path:
        sys.path.append(_p)

SCALE = 0.08838834764831845
P = 128  # partition / head-dim / token-tile size


def _build_graph(
    nt, rem, n_tokens, s_max, dma_only=False, pipeline_pv=True, replay=1
):
    """Build the SPMD Bacc graph, specialized on per-seq tile counts.

    nt[b]  = number of 128-token cache tiles for seq b
    rem[b] = valid tokens in the last tile (1..128), 0 if nt[b] == 0
    n_tokens = rows of the per-core flat cache input (B * s_max)
    s_max  = tokens per sequence in the flat cache layout
    dma_only = ablation: issue only the K/V loads (timing the memory floor)
    pipeline_pv = emit seq b's PV phase after seq b+1's score phase, so the
        exp round-trip through ScalarE doesn't stall the PE stream
    """
    import concourse.bass as bass  # noqa: F401
    import concourse.mybir as mybir
    import concourse.tile as tile
    from concourse import bacc
    from concourse.masks import make_identity

    B = len(nt)
    G = 4  # query heads per core
    MAXS = int(max(nt)) + 1  # max slots (tiles + new-token) per seq
    f32 = mybir.dt.float32
    bf16 = mybir.dt.bfloat16

    nc = bacc.Bacc(None, target_bir_lowering=False)
    kc = nc.dram_tensor("kc", [n_tokens, P], f32, kind="ExternalInput")
    vc = nc.dram_tensor("vc", [n_tokens, P], f32, kind="ExternalInput")
    qh = nc.dram_tensor("qh", [P, B * G], f32, kind="ExternalInput")  # [d,(b,g)]
    kn = nc.dram_tensor("kn", [P, B], f32, kind="ExternalInput")  # [d, b]
    vn = nc.dram_tensor("vn", [1, B * P], f32, kind="ExternalInput")  # [1,(b,d)]
    out = nc.dram_tensor("out", [B, G * P], f32, kind="ExternalOutput")

    with tile.TileContext(nc) as tc:
        with tc.tile_pool(name="persist", bufs=1) as persist:
            ident_bf = persist.tile([P, P], bf16)
            make_identity(nc, ident_bf)
            ident_f = persist.tile([P, P], f32)
            make_identity(nc, ident_f)
            ones_col_bf = persist.tile([P, 1], bf16)
            nc.vector.memset(ones_col_bf, 1.0)
            ones_row_f = persist.tile([1, P], f32)
            nc.vector.memset(ones_row_f, 1.0)
            # mask_tab[p, r] = 1.0 if p < r else 0.0 — per-partition masks for
            # the partial last tile (r = rem) and the new-token slot (r = 1)
            mask_tab = persist.tile([P, P + 1], f32)
            nc.gpsimd.memset(mask_tab, 0.0)
            # out[p, r] = (p - r >= 0) ? 0.0 (in_) : 1.0 (fill)  ==  p < r
            nc.gpsimd.affine_select(
                out=mask_tab,
                in_=mask_tab,
                compare_op=mybir.AluOpType.is_ge,
                fill=1.0,
                base=0,
                pattern=[[-1, P + 1]],
                channel_multiplier=1,
            )
            qh_bf = persist.tile([P, B * G], bf16)
            nc.gpsimd.dma_start(qh_bf[:], qh[:])
            kn_bf = persist.tile([P, B], bf16)
            nc.gpsimd.dma_start(kn_bf[:], kn[:])
            vn_bf = persist.tile([1, B * P], bf16)
            nc.gpsimd.dma_start(vn_bf[:], vn[:])
            outT = persist.tile([P, B * G], f32)  # [d, (b,g)]
            l_red = persist.tile([1, B * G], f32)
            recip = persist.tile([1, B * G], f32)
            outN = persist.tile([P, B * G], f32)
            outF = persist.tile([P, B * G], f32)

            with (
                tc.tile_pool(name="kv", bufs=3) as kvpool,
                tc.tile_pool(name="kt_ps", bufs=2, space="PSUM") as ktps,
                tc.tile_pool(name="kt_sb", bufs=3) as ktsb,
                tc.tile_pool(name="sc_ps", bufs=2, space="PSUM") as scps,
                tc.tile_pool(name="probs", bufs=2) as prpool,
                tc.tile_pool(name="ot_ps", bufs=2, space="PSUM") as otps,
                tc.tile_pool(name="l_ps", bufs=2, space="PSUM") as lps,
            ):
                state = {}

                def emit_load(b):
                    ntb = int(nt[b])
                    tok_off = b * s_max
                    kb = vb = None
                    if ntb > 0:
                        kb = kvpool.tile([P, MAXS - 1, P], bf16, tag="K")
                        vb = kvpool.tile([P, MAXS - 1, P], bf16, tag="V")
                        src_k = kc[tok_off : tok_off + ntb * P, :].rearrange(
                            "(o p) d -> p o d", p=P
                        )
                        src_v = vc[tok_off : tok_off + ntb * P, :].rearrange(
                            "(o p) d -> p o d", p=P
                        )
                        nc.gpsimd.dma_start(kb[:, :ntb, :], src_k)
                        nc.gpsimd.dma_start(vb[:, :ntb, :], src_v)
                    return kb, vb

                def emit_scores(b, kb, vb):
                    ntb = int(nt[b])
                    ns = ntb + 1
                    scores = scps.tile([P, G * MAXS], f32)
                    if b < 2:
                        # scrub pre-kernel PSUM garbage in the two rotating
                        # score buffers (rows the new-token slot never writes)
                        nc.vector.memset(scores, 0.0)
                    # transpose i+1 is emitted before QK i so the PSUM->SBUF
                    # copy of KT_i hides under transpose i+1 on the PE
                    kts_pend = []
                    for i in range(ntb):
                        ktp = ktps.tile([P, P], bf16)
                        nc.tensor.transpose(ktp, kb[:, i, :], ident_bf)
                        kts = ktsb.tile([P, P], bf16)
                        if i % 2 == 0:
                            nc.vector.tensor_copy(kts, ktp)
                        else:
                            nc.scalar.copy(kts, ktp)
                        kts_pend.append((i, kts))
                        if len(kts_pend) >= 2:
                            j, ktsj = kts_pend.pop(0)
                            nc.tensor.matmul(
                                scores[:, G * j : G * (j + 1)],
                                lhsT=ktsj,
                                rhs=qh_bf[:, G * b : G * (b + 1)],
                                start=True,
                                stop=True,
                            )
                    for j, ktsj in kts_pend:
                        nc.tensor.matmul(
                            scores[:, G * j : G * (j + 1)],
                            lhsT=ktsj,
                            rhs=qh_bf[:, G * b : G * (b + 1)],
                            start=True,
                            stop=True,
                        )
                    # new-token score (row 0 of its slot)
                    nc.tensor.matmul(
                        scores[0:1, G * ntb : G * ns],
                        lhsT=kn_bf[:, b : b + 1],
                        rhs=qh_bf[:, G * b : G * (b + 1)],
                        start=True,
                        stop=True,
                    )
                    pb = prpool.tile([P, G * MAXS], bf16)
                    nc.scalar.activation(
                        pb[:, : G * ns],
                        scores[:, : G * ns],
                        mybir.ActivationFunctionType.Exp,
                        scale=SCALE,
                    )
                    if ntb > 0 and rem[b] < P:
                        r = int(rem[b])
                        nc.vector.tensor_scalar_mul(
                            pb[:, G * (ntb - 1) : G * ntb],
                            pb[:, G * (ntb - 1) : G * ntb],
                            mask_tab[:, r : r + 1],
                        )
                    nc.vector.tensor_scalar_mul(
                        pb[:, G * ntb : G * ns],
                        pb[:, G * ntb : G * ns],
                        mask_tab[:, 1:2],
                    )
                    state[b] = (pb, vb)

                def emit_pv(b):
                    ntb = int(nt[b])
                    ns = ntb + 1
                    pb, vb = state.pop(b)
                    lp = lps.tile([1, G * MAXS], f32)
                    nc.tensor.matmul(
                        lp[:, : G * ns],
                        lhsT=ones_col_bf,
                        rhs=pb[:, : G * ns],
                        start=True,
                        stop=True,
                    )
                    otp = otps.tile([P, G], f32)
                    for i in range(ntb):
                        nc.tensor.matmul(
                            otp,
                            lhsT=vb[:, i, :],
                            rhs=pb[:, G * i : G * (i + 1)],
                            start=(i == 0),
                            stop=False,
                        )
                    nc.tensor.matmul(
                        otp,
                        lhsT=vn_bf[0:1, P * b : P * (b + 1)],
                        rhs=pb[0:1, G * ntb : G * ns],
                        start=(ntb == 0),
                        stop=True,
                    )
                    nc.vector.tensor_copy(outT[:, G * b : G * (b + 1)], otp)
                    nc.vector.tensor_reduce(
                        l_red[0:1, G * b : G * (b + 1)],
                        lp[0:1, : G * ns].rearrange("p (i h) -> p h i", h=G),
                        axis=mybir.AxisListType.X,
                        op=mybir.AluOpType.add,
                    )

                def emit_body():
                    if dma_only:
                        for b in range(B):
                            kb, vb = emit_load(b)
                            if kb is not None:
                                # tiny consumers so the loads aren't dead
                                nc.vector.tensor_copy(
                                    outT[0:1, G * b : G * b + 1],
                                    kb[0:1, 0, 0:1],
                                )
                                nc.vector.tensor_copy(
                                    outT[0:1, G * b + 1 : G * b + 2],
                                    vb[0:1, 0, 0:1],
                                )
                        nc.vector.memset(l_red, 1.0)
                    elif pipeline_pv:
                        prev = None
                        for b in range(B):
                            kb, vb = emit_load(b)
                            emit_scores(b, kb, vb)
                            if prev is not None:
                                emit_pv(prev)
                            prev = b
                        emit_pv(prev)
                    else:
                        for b in range(B):
                            kb, vb = emit_load(b)
                            emit_scores(b, kb, vb)
                            emit_pv(b)

                if replay > 1:
                    with tc.For_i(0, replay, 1):
                        emit_body()
                else:
                    emit_body()

            # ---- finalize: out = outT / l, transposed to [(b,g), d] ----
            with tc.tile_pool(name="fin_ps", bufs=1, space="PSUM") as finps:
                nc.vector.reciprocal(recip, l_red)
                bc = finps.tile([P, B * G], f32)
                nc.tensor.matmul(
                    bc, lhsT=ones_row_f, rhs=recip, start=True, stop=True
                )
                nc.vector.tensor_mul(outN, outT, bc)
                tp2 = finps.tile([P, B * G], f32)
                nc.tensor.transpose(tp2, outN, ident_f)
                nc.vector.tensor_copy(outF, tp2)
                nc.sync.dma_start(
                    out.rearrange("b (g d) -> (b g) d", g=G), outF
                )
    nc.compile()
    return nc


def _prepare(
    query, key, value, key_cache, value_cache, block_tables, seq_lens, build=True
):
    """Build the compiled SPMD graph and the per-core input shards."""
    query = np.ascontiguousarray(np.asarray(query, dtype=np.float32))
    key = np.ascontiguousarray(np.asarray(key, dtype=np.float32))
    value = np.ascontiguousarray(np.asarray(value, dtype=np.float32))
    key_cache = np.asarray(key_cache, dtype=np.float32)
    value_cache = np.asarray(value_cache, dtype=np.float32)
    block_tables = np.asarray(block_tables)
    seq_lens = np.asarray(seq_lens)

    B, H, D = query.shape
    KVH = key.shape[1]
    NB, BS = key_cache.shape[0], key_cache.shape[1]
    S_MAX = block_tables.shape[1] * BS
    G = H // KVH
    N_CORES = 8
    assert KVH == N_CORES and D == P

    L = np.maximum(seq_lens.astype(np.int64), 1)
    Lc = L - 1  # cache tokens attended (position L-1 comes from k/v_new)
    nt = ((Lc + P - 1) // P).astype(np.int64)
    rem = Lc - np.maximum(nt - 1, 0) * P  # valid tokens in last tile

    kc_flat = key_cache.reshape(NB * BS, KVH, D)
    vc_flat = value_cache.reshape(NB * BS, KVH, D)

    # The flat per-core cache is laid out seq-major: token t of seq b at row
    # b*S_MAX + t. With arange block tables (the spec's fill) that is exactly
    # the cache's own layout — a pure KV-head shard. Otherwise resolve the
    # paged layout with a host gather.
    arange_ok = bool(
        np.array_equal(
            block_tables.ravel(),
            np.arange(block_tables.size, dtype=block_tables.ravel().dtype),
        )
    )
    if not arange_ok:
        t = np.arange(S_MAX, dtype=np.int64)
        gather_idx = (
            block_tables[:, t // BS].astype(np.int64) * BS + t % BS
        ).reshape(-1)

    nc = _build_graph(nt, rem, B * S_MAX, S_MAX) if build else None

    in_maps = []
    for c in range(N_CORES):
        if arange_ok:
            kc_c = np.ascontiguousarray(kc_flat[:, c, :])
            vc_c = np.ascontiguousarray(vc_flat[:, c, :])
        else:
            kc_c = np.ascontiguousarray(kc_flat[gather_idx, c, :])
            vc_c = np.ascontiguousarray(vc_flat[gather_idx, c, :])
        qh_c = np.ascontiguousarray(
            query[:, c * G : (c + 1) * G, :].transpose(2, 0, 1).reshape(D, B * G)
        )
        kn_c = np.ascontiguousarray(key[:, c, :].T)
        vn_c = np.ascontiguousarray(value[:, c, :].reshape(1, B * D))
        in_maps.append(
            {"kc": kc_c, "vc": vc_c, "qh": qh_c, "kn": kn_c, "vn": vn_c}
        )
    return nc, in_maps, (B, H, D, G)


def kernel(query, key, value, key_cache, value_cache, block_tables, seq_lens):
    from concourse.bass_utils import run_bass_kernel_spmd

    nc, in_maps, (B, H, D, G) = _prepare(
        query, key, value, key_cache, value_cache, block_tables, seq_lens
    )
    res = run_bass_kernel_spmd(nc, in_maps, core_ids=list(range(len(in_maps))))
    out = np.empty((B, H * D), np.float32)
    for c in range(len(in_maps)):
        out[:, c * G * D : (c + 1) * G * D] = res.results[c]["out"]
    return out


# revision 27
# speedup vs baseline: 1.1309x; 1.0296x over previous
"""Paged-attention decode (GQA) on 8 Trainium2 NeuronCores.

Sharding: tensor-parallel over heads. Core c owns KV head c (KVH=8) and the
4 query heads in its GQA group. The KV cache / new K/V / query are sliced
per-core on the host (pure shard along the KV-head dim); block_tables and
seq_lens are folded into the compiled graph (decode launch config). Each core
runs an identical SPMD graph with no collectives; the host concatenates the
per-core output slices.

Device algorithm per core, per sequence b (L = seq_lens[b], Lc = L-1 cached
tokens, tiles of 128 tokens):
  - DMA K/V tiles (f32 HBM -> bf16 SBUF cast in the SWDGE datapath)
  - PE transpose each K tile -> KT [d, t]
  - scoresT[t, 4] = KT.T-matmul with qT[d, 4] (per 128-token tile, into one
    PSUM bank per sequence), plus a 1-row slot for the new token's score
  - exp(scale*s) on ACT (PSUM -> bf16 SBUF probsT), mask tail rows / new-token
    rows by memset-0 (softmax-without-max: scores are O(5), no overflow)
  - out^T[d, 4] += V_tile.T-matmul probsT tile slices, accumulated in PSUM,
    plus a rank-1 update with v_new
  - denominator l = ones-matmul over probsT, reduced per sequence on DVE
  - finalize: broadcast 1/l via a rank-1 matmul, multiply, PE-transpose to
    [(b,g), d] layout, DMA out.
"""

import numpy as np
import sys

for _p in ("/opt/trn_rl_repo",):
    if _p not in sys.path:
        sys.path.append(_p)

SCALE = 0.08838834764831845
P = 128  # partition / head-dim / token-tile size


def _build_graph(
    nt,
    rem,
    n_tokens,
    s_max,
    dma_only=False,
    pipeline_pv=True,
    replay=1,
    no_dma=False,
):
    """Build the SPMD Bacc graph, specialized on per-seq tile counts.

    nt[b]  = number of 128-token cache tiles for seq b
    rem[b] = valid tokens in the last tile (1..128), 0 if nt[b] == 0
    n_tokens = rows of the per-core flat cache input (B * s_max)
    s_max  = tokens per sequence in the flat cache layout
    dma_only = ablation: issue only the K/V loads (timing the memory floor)
    pipeline_pv = emit seq b's PV phase after seq b+1's score phase, so the
        exp round-trip through ScalarE doesn't stall the PE stream
    """
    import concourse.bass as bass  # noqa: F401
    import concourse.mybir as mybir
    import concourse.tile as tile
    from concourse import bacc
    from concourse.masks import make_identity

    B = len(nt)
    G = 4  # query heads per core
    MAXS = int(max(nt)) + 1  # max slots (tiles + new-token) per seq
    f32 = mybir.dt.float32
    bf16 = mybir.dt.bfloat16

    nc = bacc.Bacc(None, target_bir_lowering=False)
    kc = nc.dram_tensor("kc", [n_tokens, P], f32, kind="ExternalInput")
    vc = nc.dram_tensor("vc", [n_tokens, P], f32, kind="ExternalInput")
    qh = nc.dram_tensor("qh", [P, B * G], f32, kind="ExternalInput")  # [d,(b,g)]
    kn = nc.dram_tensor("kn", [P, B], f32, kind="ExternalInput")  # [d, b]
    vn = nc.dram_tensor("vn", [1, B * P], f32, kind="ExternalInput")  # [1,(b,d)]
    out = nc.dram_tensor("out", [B, G * P], f32, kind="ExternalOutput")

    with tile.TileContext(nc) as tc:
        with tc.tile_pool(name="persist", bufs=1) as persist:
            ident_bf = persist.tile([P, P], bf16)
            make_identity(nc, ident_bf)
            ident_f = persist.tile([P, P], f32)
            make_identity(nc, ident_f)
            ones_col_bf = persist.tile([P, 1], bf16)
            nc.vector.memset(ones_col_bf, 1.0)
            ones_row_f = persist.tile([1, P], f32)
            nc.vector.memset(ones_row_f, 1.0)
            # mask_tab[p, r] = 1.0 if p < r else 0.0 — per-partition masks for
            # the partial last tile (r = rem) and the new-token slot (r = 1)
            mask_tab = persist.tile([P, P + 1], f32)
            nc.gpsimd.memset(mask_tab, 0.0)
            # out[p, r] = (p - r >= 0) ? 0.0 (in_) : 1.0 (fill)  ==  p < r
            nc.gpsimd.affine_select(
                out=mask_tab,
                in_=mask_tab,
                compare_op=mybir.AluOpType.is_ge,
                fill=1.0,
                base=0,
                pattern=[[-1, P + 1]],
                channel_multiplier=1,
            )
            qh_bf = persist.tile([P, B * G], bf16)
            nc.gpsimd.dma_start(qh_bf[:], qh[:])
            kn_bf = persist.tile([P, B], bf16)
            nc.gpsimd.dma_start(kn_bf[:], kn[:])
            vn_bf = persist.tile([1, B * P], bf16)
            nc.gpsimd.dma_start(vn_bf[:], vn[:])
            outT = persist.tile([P, B * G], f32)  # [d, (b,g)]
            l_red = persist.tile([1, B * G], f32)
            recip = persist.tile([1, B * G], f32)
            outN = persist.tile([P, B * G], f32)
            outF = persist.tile([P, B * G], f32)

            if no_dma:
                dummy_k = persist.tile([P, MAXS - 1, P], bf16)
                dummy_v = persist.tile([P, MAXS - 1, P], bf16)
                nc.vector.memset(dummy_k, 0.0)
                nc.vector.memset(dummy_v, 0.0)

            with (
                tc.tile_pool(name="kv", bufs=3) as kvpool,
                tc.tile_pool(name="kt_ps", bufs=3, space="PSUM") as ktps,
                tc.tile_pool(name="kt_sb", bufs=6) as ktsb,
                tc.tile_pool(name="sc_ps", bufs=2, space="PSUM") as scps,
                tc.tile_pool(name="probs", bufs=2) as prpool,
                tc.tile_pool(name="acc_ps", bufs=3, space="PSUM") as accps,
            ):
                state = {}

                def emit_load(b):
                    ntb = int(nt[b])
                    tok_off = b * s_max
                    kb = vb = None
                    if no_dma:
                        return (dummy_k, dummy_v) if ntb > 0 else (None, None)
                    if ntb > 0:
                        kb = kvpool.tile([P, MAXS - 1, P], bf16, tag="K")
                        vb = kvpool.tile([P, MAXS - 1, P], bf16, tag="V")
                        src_k = kc[tok_off : tok_off + ntb * P, :].rearrange(
                            "(o p) d -> p o d", p=P
                        )
                        src_v = vc[tok_off : tok_off + ntb * P, :].rearrange(
                            "(o p) d -> p o d", p=P
                        )
                        nc.gpsimd.dma_start(kb[:, :ntb, :], src_k)
                        nc.gpsimd.dma_start(vb[:, :ntb, :], src_v)
                    return kb, vb

                def emit_scores(b, kb, vb):
                    ntb = int(nt[b])
                    ns = ntb + 1
                    scores = scps.tile([P, G * MAXS], f32)
                    if b < 2:
                        # scrub pre-kernel PSUM garbage in the two rotating
                        # score buffers (rows the new-token slot never writes)
                        nc.vector.memset(scores, 0.0)
                    # groups of 3: [T,T,T] then the previous group's [QK x3] —
                    # back-to-back transposes pipeline in the PE (LDWEIGHTS
                    # pull-ahead), and each PSUM->SBUF KT copy gets a full
                    # group of transposes to finish before its QK needs it
                    GRP = 3
                    kts_pend = []

                    def flush_qk(upto):
                        while len(kts_pend) > upto:
                            j, ktsj = kts_pend.pop(0)
                            nc.tensor.matmul(
                                scores[:, G * j : G * (j + 1)],
                                lhsT=ktsj,
                                rhs=qh_bf[:, G * b : G * (b + 1)],
                                start=True,
                                stop=True,
                            )

                    for i in range(ntb):
                        ktp = ktps.tile([P, P], bf16)
                        nc.tensor.transpose(ktp, kb[:, i, :], ident_bf)
                        kts = ktsb.tile([P, P], bf16)
                        if i % 2 == 0:
                            nc.vector.tensor_copy(kts, ktp)
                        else:
                            nc.scalar.copy(kts, ktp)
                        kts_pend.append((i, kts))
                        if len(kts_pend) >= 2 * GRP and i % GRP == GRP - 1:
                            flush_qk(GRP)
                    flush_qk(0)
                    # new-token score (row 0 of its slot)
                    nc.tensor.matmul(
                        scores[0:1, G * ntb : G * ns],
                        lhsT=kn_bf[:, b : b + 1],
                        rhs=qh_bf[:, G * b : G * (b + 1)],
                        start=True,
                        stop=True,
                    )
                    pb = prpool.tile([P, G * MAXS], bf16)
                    nc.scalar.activation(
                        pb[:, : G * ns],
                        scores[:, : G * ns],
                        mybir.ActivationFunctionType.Exp,
                        scale=SCALE,
                    )
                    if ntb > 0 and rem[b] < P:
                        r = int(rem[b])
                        nc.vector.tensor_scalar_mul(
                            pb[:, G * (ntb - 1) : G * ntb],
                            pb[:, G * (ntb - 1) : G * ntb],
                            mask_tab[:, r : r + 1],
                        )
                    nc.vector.tensor_scalar_mul(
                        pb[:, G * ntb : G * ns],
                        pb[:, G * ntb : G * ns],
                        mask_tab[:, 1:2],
                    )
                    state[b] = (pb, vb)

                def emit_pv(b):
                    ntb = int(nt[b])
                    ns = ntb + 1
                    pb, vb = state.pop(b)
                    lp = accps.tile([1, G * MAXS], f32, tag="acc")
                    nc.tensor.matmul(
                        lp[:, : G * ns],
                        lhsT=ones_col_bf,
                        rhs=pb[:, : G * ns],
                        start=True,
                        stop=True,
                    )
                    otp = accps.tile([P, G], f32, tag="acc")
                    for i in range(ntb):
                        nc.tensor.matmul(
                            otp,
                            lhsT=vb[:, i, :],
                            rhs=pb[:, G * i : G * (i + 1)],
                            start=(i == 0),
                            stop=False,
                        )
                    nc.tensor.matmul(
                        otp,
                        lhsT=vn_bf[0:1, P * b : P * (b + 1)],
                        rhs=pb[0:1, G * ntb : G * ns],
                        start=(ntb == 0),
                        stop=True,
                    )
                    nc.vector.tensor_copy(outT[:, G * b : G * (b + 1)], otp)
                    nc.vector.tensor_reduce(
                        l_red[0:1, G * b : G * (b + 1)],
                        lp[0:1, : G * ns].rearrange("p (i h) -> p h i", h=G),
                        axis=mybir.AxisListType.X,
                        op=mybir.AluOpType.add,
                    )

                def emit_body():
                    if dma_only:
                        for b in range(B):
                            kb, vb = emit_load(b)
                            if kb is not None:
                                # tiny consumers so the loads aren't dead
                                nc.vector.tensor_copy(
                                    outT[0:1, G * b : G * b + 1],
                                    kb[0:1, 0, 0:1],
                                )
                                nc.vector.tensor_copy(
                                    outT[0:1, G * b + 1 : G * b + 2],
                                    vb[0:1, 0, 0:1],
                                )
                        nc.vector.memset(l_red, 1.0)
                    elif pipeline_pv:
                        prev = None
                        for b in range(B):
                            kb, vb = emit_load(b)
                            emit_scores(b, kb, vb)
                            if prev is not None:
                                emit_pv(prev)
                            prev = b
                        emit_pv(prev)
                    else:
                        for b in range(B):
                            kb, vb = emit_load(b)
                            emit_scores(b, kb, vb)
                            emit_pv(b)

                if replay > 1:
                    with tc.For_i(0, replay, 1):
                        emit_body()
                else:
                    emit_body()

            # ---- finalize: out = outT / l, transposed to [(b,g), d] ----
            with tc.tile_pool(name="fin_ps", bufs=1, space="PSUM") as finps:
                nc.vector.reciprocal(recip, l_red)
                bc = finps.tile([P, B * G], f32)
                nc.tensor.matmul(
                    bc, lhsT=ones_row_f, rhs=recip, start=True, stop=True
                )
                nc.vector.tensor_mul(outN, outT, bc)
                tp2 = finps.tile([P, B * G], f32)
                nc.tensor.transpose(tp2, outN, ident_f)
                nc.vector.tensor_copy(outF, tp2)
                nc.sync.dma_start(
                    out.rearrange("b (g d) -> (b g) d", g=G), outF
                )
    nc.compile()
    return nc


def _prepare(
    query, key, value, key_cache, value_cache, block_tables, seq_lens, build=True
):
    """Build the compiled SPMD graph and the per-core input shards."""
    query = np.ascontiguousarray(np.asarray(query, dtype=np.float32))
    key = np.ascontiguousarray(np.asarray(key, dtype=np.float32))
    value = np.ascontiguousarray(np.asarray(value, dtype=np.float32))
    key_cache = np.asarray(key_cache, dtype=np.float32)
    value_cache = np.asarray(value_cache, dtype=np.float32)
    block_tables = np.asarray(block_tables)
    seq_lens = np.asarray(seq_lens)

    B, H, D = query.shape
    KVH = key.shape[1]
    NB, BS = key_cache.shape[0], key_cache.shape[1]
    S_MAX = block_tables.shape[1] * BS
    G = H // KVH
    N_CORES = 8
    assert KVH == N_CORES and D == P

    L = np.maximum(seq_lens.astype(np.int64), 1)
    Lc = L - 1  # cache tokens attended (position L-1 comes from k/v_new)
    nt = ((Lc + P - 1) // P).astype(np.int64)
    rem = Lc - np.maximum(nt - 1, 0) * P  # valid tokens in last tile

    kc_flat = key_cache.reshape(NB * BS, KVH, D)
    vc_flat = value_cache.reshape(NB * BS, KVH, D)

    # The flat per-core cache is laid out seq-major: token t of seq b at row
    # b*S_MAX + t. With arange block tables (the spec's fill) that is exactly
    # the cache's own layout — a pure KV-head shard. Otherwise resolve the
    # paged layout with a host gather.
    arange_ok = bool(
        np.array_equal(
            block_tables.ravel(),
            np.arange(block_tables.size, dtype=block_tables.ravel().dtype),
        )
    )
    if not arange_ok:
        t = np.arange(S_MAX, dtype=np.int64)
        gather_idx = (
            block_tables[:, t // BS].astype(np.int64) * BS + t % BS
        ).reshape(-1)

    nc = _build_graph(nt, rem, B * S_MAX, S_MAX) if build else None

    in_maps = []
    for c in range(N_CORES):
        if arange_ok:
            kc_c = np.ascontiguousarray(kc_flat[:, c, :])
            vc_c = np.ascontiguousarray(vc_flat[:, c, :])
        else:
            kc_c = np.ascontiguousarray(kc_flat[gather_idx, c, :])
            vc_c = np.ascontiguousarray(vc_flat[gather_idx, c, :])
        qh_c = np.ascontiguousarray(
            query[:, c * G : (c + 1) * G, :].transpose(2, 0, 1).reshape(D, B * G)
        )
        kn_c = np.ascontiguousarray(key[:, c, :].T)
        vn_c = np.ascontiguousarray(value[:, c, :].reshape(1, B * D))
        in_maps.append(
            {"kc": kc_c, "vc": vc_c, "qh": qh_c, "kn": kn_c, "vn": vn_c}
        )
    return nc, in_maps, (B, H, D, G)


def kernel(query, key, value, key_cache, value_cache, block_tables, seq_lens):
    from concourse.bass_utils import run_bass_kernel_spmd

    nc, in_maps, (B, H, D, G) = _prepare(
        query, key, value, key_cache, value_cache, block_tables, seq_lens
    )
    res = run_bass_kernel_spmd(nc, in_maps, core_ids=list(range(len(in_maps))))
    out = np.empty((B, H * D), np.float32)
    for c in range(len(in_maps)):
        out[:, c * G * D : (c + 1) * G * D] = res.results[c]["out"]
    return out
